# revision 43
# baseline (speedup 1.0000x reference)
"""v3: paired exp (one 1792-col ACT inst per q-block over a psum slot pair),
f16 PE transposes (Pool pre-casts f32->f16), DVE computes 2/16 k-tiles of exp
via 1-pass int16 Schraudolph, O lags S by 2 pairs (PT triple-buffered).

PSUM layout (f32 cols): slots 0..2 at 0/1024/2048 (S logits, rotation),
opsum bufs at 3072+{0,512} (129 cols each: 128 d + ones-column denominator).
Steady-state transposes write f16 tile slots at f32 cols 192..512 of the
opsum buf of parity (w+1)%2; bank-level PE-write/DVE-read exclusion is
enforced by gates: O(p) waits batch-(p+1) copies, window-w transposes wait
norm(w-3) and batch-(w-2) copies.
"""
import os
import numpy as np
import concourse.bass as bass
from concourse import mybir
from contextlib import ExitStack

F32 = mybir.dt.float32
F16 = mybir.dt.float16
I16 = mybir.dt.int16
I32 = mybir.dt.int32
EXP = mybir.ActivationFunctionType.Exp
SCALE = float(1.0 / np.sqrt(128.0))
LN2 = float(np.log(2.0))
A16 = (1 << 10) / LN2 * SCALE
B16 = 15.0 * (1 << 10) - 61.0
A32 = (1 << 23) / LN2 * SCALE
B32 = 127.0 * (1 << 23) - 500000.0

N_CORES = 8
N_WARM = 40
LABELS = {}


def _lab(inst, label):
    try:
        LABELS[inst.ins.name] = label
    except Exception:
        pass
    return inst


def build_attention_nc(SEQ=2048, B=2, G=4):
    NO_DUMMY = bool(int(os.environ.get("BIS_NO_DUMMY", "0")))
    NO_SPLIT = bool(int(os.environ.get("BIS_NO_SPLIT", "0")))
    NO_EXPD = bool(int(os.environ.get("BIS_NO_EXPD", "0")))
    EXPD_DUMMY = bool(int(os.environ.get("BIS_EXPD_DUMMY", "0")))
    EXPD_N = int(os.environ.get("BIS_EXPD_N", "128"))
    EXPD_NOGATE = bool(int(os.environ.get("BIS_EXPD_NOGATE", "0")))
    def expd_on(p):
        return (not NO_EXPD) and p < EXPD_N
    ACT_W = 1024 if (NO_EXPD or EXPD_DUMMY) else 768
    D = 128
    T = SEQ // 128            # 16 k/q tiles per head
    H = B * G                 # 8 (b, g) heads per core
    NPH = T                   # pairs (q-blocks) per head
    NPAIR = H * NPH           # 128
    NW = NPAIR + 2            # windows (O lags 2)
    assert T == 16 and H == 8 and B == 2

    nc = bass.Bass()
    q_ext = nc.declare_dram_parameter("query", [SEQ, B, G, D], F32, isOutput=False)
    k_ext = nc.declare_dram_parameter("key", [SEQ, B, D], F32, isOutput=False)
    v_ext = nc.declare_dram_parameter("value", [SEQ, B, D], F32, isOutput=False)
    o_ext = nc.declare_dram_parameter("out", [SEQ, B, G, D], F32, isOutput=True)

    # loads in first-use order: K(b0), Q(h0..h3), K(b1), Q(h4..h7)
    loads = [("K", 0, None)] + [("Q", 0, g) for g in range(G)]
    loads += [("K", 1, None)] + [("Q", 1, g) for g in range(G)]
    NL = len(loads)           # 10

    def q_load_index(h):
        b, g = divmod(h, G)
        return b * (G + 1) + 1 + g

    # steady-state transpose windows per load (prologue covers l0 all + l1 t0)
    due_w = {}
    per_w = {}

    trs_in_w = {w: [] for w in range(NW)}
    for i, t in enumerate(range(1, T)):       # l1 (Q h0) tiles 1..15
        trs_in_w[i // 2].append((1, t))
    for ld, ws in due_w.items():
        n = per_w.get(ld, 2)
        t = 0
        for w in ws:
            for _ in range(n):
                if t < T:
                    trs_in_w[w].append((ld, t))
                    t += 1
        assert t >= T, (ld, t)
    for w, trs in trs_in_w.items():
        assert len(trs) <= 5

    # cast (Pool f32->f16) events: (load, t0, t1, sem_load threshold)
    # (load, t0, t1, chunk): chunk 0/1 for split loads 0,1; else 0
    if NO_SPLIT:
        cast_events = [(ld, 0, 16, 0) for ld in range(NL)]
    else:
        cast_events = [(0, 0, 8, 0), (1, 0, 1, 0), (0, 8, 16, 1),
                       (1, 1, 8, 0), (1, 8, 16, 1)]
        cast_events += [(ld, 0, 16, 0) for ld in range(2, NL)]

    def cast_val_for(ld, t):
        for i, (l, t0, t1, _c) in enumerate(cast_events):
            if l == ld and t0 <= t < t1:
                return i + 1
        raise AssertionError((ld, t))

    # ---------------- schedule walk (mirrors emission exactly) ----------
    pe = 0
    pe_after_S = {}
    pe_after_O = {}
    pe_after_tr = {}
    load_last_pe = {}

    def note_tr(ld, t):
        nonlocal pe
        pe += 1
        pe_after_tr[(ld, t)] = pe
        load_last_pe[ld] = max(load_last_pe.get(ld, 0), pe)

    # prologue: l0 t0-7 -> bank6 s0-7; l1 t0 -> bank7 s0; S(0);
    # l0 t8-15 -> bank7 s1-7 + bank6 s0; S(1); then windows w>=0 trs/O.
    for t in range(8):
        note_tr(0, t)
    note_tr(1, 0)
    pe += 8
    pe_after_S[0] = pe
    for t in range(8, 16):
        note_tr(0, t)
    pe += 8
    pe_after_S[1] = pe
    for (ld, t) in trs_in_w.get(0, []):
        note_tr(ld, t)
    for w in range(NW):
        if w < NPAIR and w >= 1:
            pe += 8
            pe_after_S[2 * w] = pe
        if w >= 1:
            for (ld, t) in trs_in_w.get(w, []):
                note_tr(ld, t)
        if w >= 2:
            pe += 16
            pe_after_O[w - 2] = pe
        if w < NPAIR and w >= 1:
            pe += 8
            pe_after_S[2 * w + 1] = pe

    # DVE stream walk
    dve = 0
    copy_done = {}
    copy_batch_done = {}
    dve_exp_done = {}
    tsa_done = {}
    tsb_done = {}
    recips_done = {}
    mults_done = {}
    dve_ops = []

    def batch_runs(w):
        trs = trs_in_w.get(w, [])
        runs = []
        for (ld, t) in trs:
            if runs and runs[-1][0] == ld and runs[-1][2] == t:
                runs[-1] = [ld, runs[-1][1], t + 1]
            else:
                runs.append([ld, t, t + 1])
        return runs

    def note_copy_batch(w):
        nonlocal dve
        trs = trs_in_w.get(w, [])
        if not trs:
            copy_batch_done[w] = 0
            return
        runs = batch_runs(w)
        dve += len(runs)
        for (ld, t) in trs:
            copy_done[(ld, t)] = dve
        copy_batch_done[w] = dve
        dve_ops.append(("copies", w, runs))

    for kind, keys in [("b6", [(0, t) for t in range(8)]),
                       ("l1t0", [(1, 0)]),
                       ("b7a", [(0, t) for t in range(8, 15)]),
                       ("l0t15", [(0, 15)])]:
        dve += 1
        for kk in keys:
            copy_done[kk] = dve
        dve_ops.append(("pcopy", kind))

    for w in range(NW):
        if w < NPAIR and expd_on(w):
            dve += 1
            tsa_done[w] = dve
            dve_ops.append(("expts", w))
        note_copy_batch(w)
        if w >= 2:
            p = w - 2
            dve += 1
            recips_done[p] = dve
            dve_ops.append(("recip", p))
            dve += 1
            mults_done[p] = dve
            dve_ops.append(("mult", p))
        if w < NPAIR and expd_on(w):
            dve += 1
            tsb_done[w] = dve
            dve_exp_done[w] = dve
            dve_ops.append(("expcp", w))

    head_ready = {}

    # PT half mapping: halves ordered by ascending psum slot address
    pt_half = {}
    for p in range(NPAIR):
        if p % 3 == 1:     # slots (2,0) -> ascending (0,2): half0 = group 2p+1
            pt_half[2 * p] = 1
            pt_half[2 * p + 1] = 0
        else:
            pt_half[2 * p] = 0
            pt_half[2 * p + 1] = 1

    # ---------------- tensors ----------------
    ident = nc.alloc_sbuf_tensor("ident", [128, 128], F16)
    warm = nc.alloc_sbuf_tensor("warm", [128, 128], F16)
    bias0 = nc.alloc_sbuf_tensor("bias0", [128, 1], F32)
    scr = nc.alloc_sbuf_tensor("scr", [128, 1], F32)
    scr32 = nc.alloc_sbuf_tensor("scr32", [128, 1024], I32)
    scrPT = nc.alloc_sbuf_tensor("scrPT", [128, 512], F16)
    qnat = [nc.alloc_sbuf_tensor(f"qnat{i}", [128, T * 128], F32) for i in range(3)]
    qnat16 = [nc.alloc_sbuf_tensor(f"qnat16_{i}", [128, T * 128], F16)
              for i in range(3)]
    KT = [nc.alloc_sbuf_tensor(f"KT{b}", [128, T * 128], F16) for b in range(B)]
    QT = [nc.alloc_sbuf_tensor(f"QT{h}", [128, T * 128], F16) for h in range(H)]
    VT = [nc.alloc_sbuf_tensor(f"VT{b}", [128, T * 132], F16) for b in range(B)]
    PT = [nc.alloc_sbuf_tensor(f"PT{s}", [128, 2048], F16) for s in range(3)]
    rsb = [nc.alloc_sbuf_tensor(f"rsb{s}", [128, 1], F32) for s in range(2)]
    OS = [nc.alloc_sbuf_tensor(f"OS{s}", [128, T * 128], F32) for s in range(2)]
    psum = nc.alloc_psum_tensor("psum", [128, 4096], F32)

    pv = psum[:, :].rearrange("p (s c) -> p s c", c=1024)
    PTI = [PT[s][:, :].bitcast(I16) for s in range(3)]
    p16 = psum[:].bitcast(F16)           # [128, 8192]

    def spsum_mm(slot, ki):
        return psum[:, slot * 1024 + ki * 128: slot * 1024 + (ki + 1) * 128]

    def opsum(buf):
        off = 3072 + buf * 512
        return psum[:, off:off + 129]

    def exp_slots(p, c0, c1):
        r = p % 3
        if r == 0:
            return pv[:, 0:2, c0:c1]
        if r == 1:
            return pv[:, 0::2, c0:c1]
        return pv[:, 1:3, c0:c1]

    def tr_parity(w):
        return (w + 1) % 2

    def tr_psum_w(w, k):
        base = (6 + tr_parity(w)) * 1024 + 384 + k * 128
        return p16[:, base:base + 128]

    PRO_SLOT = {}       # (ld,t) -> f16 col base for prologue trs
    for t in range(8):
        PRO_SLOT[(0, t)] = 6 * 1024 + t * 128
    PRO_SLOT[(1, 0)] = 7 * 1024
    for t in range(8, 15):
        PRO_SLOT[(0, t)] = 7 * 1024 + (t - 7) * 128
    PRO_SLOT[(0, 15)] = 6 * 1024

    with ExitStack() as ctx:
        sem_pe = ctx.enter_context(nc.semaphore("sem_pe"))
        sem_act = ctx.enter_context(nc.semaphore("sem_act"))
        sem_dve = ctx.enter_context(nc.semaphore("sem_dve"))
        sem_cast = ctx.enter_context(nc.semaphore("sem_cast"))
        sem_pool = ctx.enter_context(nc.semaphore("sem_pool"))
        sem_load = {}
        for i in range(NL):
            nch = 1 if NO_SPLIT else (2 if i < 2 else 1)
            for c in range(nch):
                sem_load[(i, c)] = ctx.enter_context(
                    nc.semaphore(f"sem_load{i}_{c}"))
        sem_out = [ctx.enter_context(nc.semaphore(f"sem_out{h}"))
                   for h in range(H)]
        sem_str = {i: ctx.enter_context(nc.semaphore(f"sem_str{i}"))
                   for i in range(2, NL)}
        sem_v = [ctx.enter_context(nc.semaphore(f"sem_v{b}")) for b in range(B)]
        block = ctx.enter_context(nc.Block())

        @block.sync
        def _(sync):
            def ld_src(i):
                kind, b, g = loads[i]
                return k_ext[:, b, :] if kind == "K" else q_ext[:, b, g, :]

            def emit_load(i, t0, t1, chunk):
                src = ld_src(i).rearrange("(t p) d -> p t d", p=128)
                dst = qnat[i % 3][:].rearrange("p (t d) -> p t d", d=128)
                nc.sync.dma_start(
                    out=dst[:, t0:t1, :], in_=src[:, t0:t1, :],
                ).then_inc(sem_load[(i, chunk)], 16)

            def emit_xbar(i):
                kind, b, g = loads[i]
                tt = KT[b] if kind == "K" else QT[b * G + g]
                nc.sync.wait_ge(sem_cast, cast_val_for(i, 0))
                dst = tt[:].rearrange("p (t d) -> p t d", d=128)
                _lab(nc.sync.dma_start_transpose(
                    dst[:, 0:16, :],
                    qnat16[i % 3][:, 0:T * 128],
                ), f"X(l{i})").then_inc(sem_str[i], 16)

            emit_load(0, 0, 8, 0)
            emit_load(1, 0, 8, 0)
            emit_load(0, 8, 16, 1)
            emit_load(1, 8, 16, 1)
            for i in range(2, NL):
                if i == 3 or i == 4:
                    nc.sync.wait_ge(sem_pe, load_last_pe[i - 3])
                elif i >= 5:
                    nc.sync.wait_ge(sem_str[i - 3], 16)
                emit_load(i, 0, 16, 0)
                if i >= 4:
                    emit_xbar(i - 2)
            emit_xbar(8)
            emit_xbar(9)
            for h in range(H):
                nc.sync.wait_ge(sem_out[h], 64 if h == H - 1 else 32)

        @block.gpsimd
        def _(gp):
            nc.gpsimd.memset(warm[:], 0.0).then_inc(sem_pool)
            nc.gpsimd.memset(ident[:], 0.0).then_inc(sem_pool)
            nc.gpsimd.wait_ge(sem_pool, 2)
            nc.gpsimd.affine_select(
                out=ident[:], in_=ident[:],
                compare_op=mybir.AluOpType.not_equal, fill=1.0,
                base=0, pattern=[[-1, 128]], channel_multiplier=1,
            ).then_inc(sem_pool)
            nc.gpsimd.memset(bias0[:], 0.0).then_inc(sem_pool)
            for b in range(B):
                vt3 = VT[b][:].rearrange("p (t c) -> p t c", c=132)
                nc.gpsimd.memset(vt3[:, :, 128:129], 1.0).then_inc(sem_pool)
                nc.gpsimd.dma_start(
                    out=vt3[:, :, 0:128],
                    in_=v_ext[:, b, :].rearrange("(t p) d -> p t d", p=128),
                ).then_inc(sem_v[b], 16)

            def cast(idx):
                ld, t0, t1, chunk = cast_events[idx]
                nc.gpsimd.wait_ge(sem_load[(ld, chunk)], 16)
                nc.gpsimd.tensor_copy(
                    qnat16[ld % 3][:, t0 * 128:t1 * 128],
                    qnat[ld % 3][:, t0 * 128:t1 * 128],
                ).then_inc(sem_cast)

            def out_store(h, half):
                b, g = divmod(h, G)
                oh = o_ext[:, b, g, :].rearrange("(t p) d -> p t d", p=128)
                osh = OS[h % 2][:].rearrange("p (t d) -> p t d", d=128)
                if h == H - 1:
                    for q in (0, 1) if half == 0 else (2, 3):
                        p_end = h * NPH + 4 * q + 3
                        nc.gpsimd.wait_ge(sem_dve, mults_done[p_end])
                        nc.gpsimd.dma_start(
                            out=oh[:, 4 * q:4 * q + 4, :],
                            in_=osh[:, 4 * q:4 * q + 4, :],
                        ).then_inc(sem_out[h], 16)
                    return
                hf = T // 2
                p_end = h * NPH + (half + 1) * hf - 1
                nc.gpsimd.wait_ge(sem_dve, mults_done[p_end])
                nc.gpsimd.dma_start(
                    out=oh[:, half * hf:(half + 1) * hf, :],
                    in_=osh[:, half * hf:(half + 1) * hf, :],
                ).then_inc(sem_out[h], 16)

            # c0..c4: l0/l1 chunks; c5..c12: loads 2..9
            if NO_SPLIT:
                order = ["c0", "c1", "c2", "c3", "c4", "o0a", "c5",
                         "o0b", "o1a", "c6", "o1b", "o2a", "c7", "o2b",
                         "o3a", "c8", "o3b", "c9", "o4a", "o4b",
                         "o5a", "o5b", "o6a", "o6b", "o7a", "o7b"]
            else:
                order = ["c0", "c1", "c2", "c3", "c4", "c5", "c6",
                         "c7", "o0a", "c8", "o0b", "o1a", "c9", "o1b",
                         "o2a", "c10", "o2b", "o3a", "c11", "o3b",
                         "c12", "o4a", "o4b", "o5a", "o5b",
                         "o6a", "o6b", "o7a", "o7b"]
            for op in order:
                if op[0] == "c":
                    cast(int(op[1:]))
                else:
                    out_store(int(op[1]), 0 if op[2] == "a" else 1)

        @block.tensor
        def _(te):
            done_str = set()

            def str_wait(i):
                if i not in done_str:
                    done_str.add(i)
                    nc.tensor.wait_ge(sem_str[i], 16)

            nc.tensor.wait_ge(sem_pool, 1)
            for i in range(N_WARM):
                _lab(nc.tensor.matmul(
                    psum[:, 2048:2176], warm[:], warm[:],
                    start=True, stop=True, skip_group_check=True,
                ), f"warm{i}")
            nc.tensor.wait_ge(sem_pool, 3)
            seen_cast = set()
            last_dve_wait = [0]

            def twait(val):
                if val > last_dve_wait[0]:
                    last_dve_wait[0] = val
                    nc.tensor.wait_ge(sem_dve, val)

            def emit_tr(ld, t, dst):
                cv = cast_val_for(ld, t)
                if cv not in seen_cast:
                    seen_cast.add(cv)
                    nc.tensor.wait_ge(sem_cast, cv)
                _lab(nc.tensor.transpose(
                    dst, qnat16[ld % 3][:, t * 128:(t + 1) * 128], ident[:],
                ), f"tr(l{ld},t{t})").then_inc(sem_pe)

            def emit_S(g):
                p = g >> 1
                h = p // NPH
                slot = g % 3
                kp = g & 1
                b = h // G
                qc = p % NPH
                if g == 2 * h * NPH and h >= 1:
                    if h >= G:
                        str_wait(5)
                    str_wait(q_load_index(h))
                if h == 0 and qc > 0 and kp == 0:
                    twait(copy_done[(1, qc)])
                if g == 0:
                    twait(copy_done[(1, 0)])
                if g == 1:
                    twait(copy_done[(0, 15)])
                for ki in range(8):
                    kt = kp * 8 + ki
                    inst = nc.tensor.matmul(
                        spsum_mm(slot, ki),
                        KT[b][:, kt * 128:(kt + 1) * 128],
                        QT[h][:, qc * 128:(qc + 1) * 128],
                        start=True, stop=True, skip_group_check=True,
                    )
                    if ki == 0 and g >= 3:
                        inst._wait_ge(sem_act, g - 2)
                    _lab(inst, f"S(g{g},ki{ki})")
                    inst.then_inc(sem_pe)

            def emit_O(p):
                h = p // NPH
                b = h // G
                buf = p % 2
                if p == 0 or p == G * NPH:
                    nc.tensor.wait_ge(sem_v[b], 16)
                    nc.tensor.wait_ge(sem_pool, 5 + b)
                w_gate = 0
                if p >= 2:
                    w_gate = mults_done[p - 2]        # opsum buf reuse
                w_gate = max(w_gate, copy_batch_done.get(p + 1, 0))  # bank P10
                if expd_on(p):
                    w_gate = max(w_gate, tsb_done[p])
                if expd_on(p + 1):
                    w_gate = max(w_gate, tsa_done[p + 1])
                if w_gate:
                    twait(w_gate)
                vt3 = VT[b][:].rearrange("p (t c) -> p t c", c=132)
                kts = [0, 1, 2, 3, 4] + list(range(8, 16)) + [5, 6, 7]
                for i, kt in enumerate(kts):
                    g = 2 * p + (kt // 8)
                    half = pt_half[g]
                    ki = kt % 8
                    inst = nc.tensor.matmul(
                        opsum(buf),
                        PT[p % 3][:, half * 1024 + ki * 128:
                                  half * 1024 + (ki + 1) * 128],
                        vt3[:, kt, 0:129],
                        start=(i == 0), stop=(i == len(kts) - 1),
                        skip_group_check=True,
                    )
                    if i == 0:
                        inst._wait_ge(sem_act, 2 * p + 2)
                    _lab(inst, f"O(p{p},kt{kt})")
                    inst.then_inc(sem_pe)

            # prologue
            for t in range(8):
                emit_tr(0, t, p16[:, PRO_SLOT[(0, t)]:PRO_SLOT[(0, t)] + 128])
            emit_tr(1, 0, p16[:, PRO_SLOT[(1, 0)]:PRO_SLOT[(1, 0)] + 128])
            emit_S(0)
            for t in range(8, 16):
                if t == 15:
                    twait(copy_done[(0, 0)])   # bank6 s0 reuse
                emit_tr(0, t, p16[:, PRO_SLOT[(0, t)]:PRO_SLOT[(0, t)] + 128])
            emit_S(1)
            for k, (ld, t) in enumerate(trs_in_w.get(0, [])):
                emit_tr(ld, t, tr_psum_w(0, k))

            for w in range(NW):
                if w < NPAIR and w >= 1:
                    emit_S(2 * w)
                trs = trs_in_w.get(w, []) if w >= 1 else []
                if trs:
                    twait(copy_done[(0, 15)])          # all prologue copies
                    if w >= 3:
                        twait(mults_done[w - 3])       # norm done (same bank)
                    if w >= 2:
                        twait(copy_batch_done.get(w - 2, 0))  # slot reuse
                    for k, (ld, t) in enumerate(trs):
                        emit_tr(ld, t, tr_psum_w(w, k))
                if w >= 2:
                    emit_O(w - 2)
                if w < NPAIR and w >= 1:
                    emit_S(2 * w + 1)

        @block.scalar
        def _(sc):
            nc.scalar.wait_ge(sem_pool, 4)
            if not NO_DUMMY:
                nc.scalar.activation(                  # preload Exp table
                    out=scr[:, 0:1], in_=bias0[:, 0:1],
                    func=EXP, bias=bias0[:, 0:1], scale=1.0,
                )
            for p in range(NPAIR):
                ov = PT[p % 3][:, :].rearrange("p (s c) -> p s c", c=1024)
                for pos in range(2):
                    g = 2 * p + pos
                    slot = g % 3
                    half = pt_half[g]
                    aw = 512 if (expd_on(p) and pos == 0) else 1024
                    _lab(nc.scalar.activation(
                        out=ov[:, half:half + 1, 0:aw],
                        in_=pv[:, slot:slot + 1, 0:aw],
                        func=EXP, bias=bias0[:, 0:1], scale=SCALE,
                    )._wait_ge(sem_pe, pe_after_S[g]),
                        f"exp(g{g})").then_inc(sem_act)

        @block.vector
        def _(ve):
            def emit_pcopy(kind):
                if kind == "b6":
                    nc.vector.wait_ge(sem_pe, pe_after_tr[(0, 7)])
                    nc.vector.tensor_copy(
                        KT[0][:, 0:1024],
                        p16[:, 6 * 1024:6 * 1024 + 1024]).then_inc(sem_dve)
                elif kind == "l1t0":
                    nc.vector.wait_ge(sem_pe, pe_after_tr[(1, 0)])
                    nc.vector.tensor_copy(
                        QT[0][:, 0:128],
                        p16[:, 7 * 1024:7 * 1024 + 128]).then_inc(sem_dve)
                elif kind == "b7a":
                    nc.vector.wait_ge(sem_pe, pe_after_tr[(0, 14)])
                    nc.vector.tensor_copy(
                        KT[0][:, 1024:1920],
                        p16[:, 7 * 1024 + 128:8 * 1024]).then_inc(sem_dve)
                else:  # l0t15
                    nc.vector.wait_ge(sem_pe, pe_after_tr[(0, 15)])
                    nc.vector.tensor_copy(
                        KT[0][:, 1920:2048],
                        p16[:, 6 * 1024:6 * 1024 + 128]).then_inc(sem_dve)

            def emit_copies(w, runs):
                trs = trs_in_w[w]
                nc.vector.wait_ge(sem_pe, pe_after_tr[tuple(trs[-1])])
                k = 0
                base = (6 + tr_parity(w)) * 1024 + 384
                for (ld, t0, t1) in runs:
                    n = t1 - t0
                    src = p16[:, base + k * 128:base + (k + n) * 128]
                    kind, b, g = loads[ld]
                    tt = KT[b] if kind == "K" else QT[b * G + g]
                    _lab(nc.vector.tensor_copy(
                        tt[:, t0 * 128:t1 * 128], src), f"cp(w{w},l{ld},t{t0}-{t1})").then_inc(sem_dve)
                    k += n

            def emit_expts(p):
                g = 2 * p
                slot = g % 3
                nc.vector.wait_ge(sem_pe, pe_after_S[g])
                off = (p % 2) * 512
                _lab(nc.vector.tensor_scalar(
                    scr32[:, off:off + 512], pv[:, slot, 512:1024],
                    A32, B32, op0=mybir.AluOpType.mult,
                    op1=mybir.AluOpType.add,
                ), f"expDts({p})").then_inc(sem_dve)

            def emit_expcp(p):
                half = pt_half[2 * p]
                off = (p % 2) * 512
                nc.vector.wait_ge(sem_dve, tsa_done[p])   # scr32 RAW drain
                _lab(nc.vector.tensor_copy(
                    PT[p % 3][:, half * 1024 + 512:half * 1024 + 1024],
                    scr32[:, off:off + 512].bitcast(F32),
                ), f"expDcp({p})").then_inc(sem_dve)

            def emit_recip(p):
                buf = p % 2
                nc.vector.wait_ge(sem_pe, pe_after_O[p])
                if p >= 2:
                    nc.vector.wait_ge(sem_dve, mults_done[p - 2])
                _lab(nc.vector.reciprocal(
                    rsb[buf][:, 0:1], opsum(buf)[:, 128:129]), f"recip({p})").then_inc(sem_dve)

            def emit_mult(p):
                h = p // NPH
                qc = p % NPH
                buf = p % 2
                nc.vector.wait_ge(sem_dve, recips_done[p])
                if qc == 0 and h >= 2:
                    nc.vector.wait_ge(sem_out[h - 2], 32)
                _lab(nc.vector.tensor_scalar(
                    OS[h % 2][:, qc * 128:(qc + 1) * 128],
                    opsum(buf)[:, 0:128],
                    rsb[buf][:, 0:1],
                    None,
                    op0=mybir.AluOpType.mult,
                ), f"mult({p})").then_inc(sem_dve)

            n_pad = int(os.environ.get("BIS_DVE_PAD", "0"))
            pad_kind = os.environ.get("BIS_PAD_KIND", "tiny")
            for i in range(n_pad):
                if pad_kind == "tiny":
                    nc.vector.tensor_copy(scr[:, 0:1], bias0[:, 0:1])
                elif pad_kind == "ts_psum":
                    nc.vector.tensor_scalar(
                        scr32[:, :].rearrange("p (s c) -> p s c", c=256)[:, 0:2, :],
                        pv[:, 0::2, 768:1024],
                        A32, B32, op0=mybir.AluOpType.mult,
                        op1=mybir.AluOpType.add)
                elif pad_kind == "ts_sbuf":
                    nc.vector.tensor_scalar(
                        scr32[:, 0:256], scrPT[:, 0:256].bitcast(F32).to_broadcast([128, 256]) if False else scr32[:, 256:512].bitcast(F32),
                        A32, B32, op0=mybir.AluOpType.mult,
                        op1=mybir.AluOpType.add)
                elif pad_kind == "cp_big":
                    nc.vector.tensor_copy(
                        scrPT[:, :].rearrange("p (s c) -> p s c", c=256)[:, 0:2, :],
                        scr32[:].bitcast(F32).rearrange(
                            "p (s c) -> p s c", c=256)[:, 0:2, :])
            for op in dve_ops:
                if op[0] == "pcopy":
                    emit_pcopy(op[1])
                elif op[0] == "copies":
                    emit_copies(op[1], op[2])
                elif op[0] == "expts":
                    emit_expts(op[1])
                elif op[0] == "expcp":
                    emit_expcp(op[1])
                elif op[0] == "recip":
                    emit_recip(op[1])
                else:
                    emit_mult(op[1])

    return nc


_NC = None


def _get_nc():
    global _NC
    if _NC is None:
        _NC = build_attention_nc(2048, 2, 4)
    return _NC


def kernel(query, key, value):
    from concourse.bass_utils import run_bass_kernel_spmd

    query = np.ascontiguousarray(query, dtype=np.float32)
    key = np.ascontiguousarray(key, dtype=np.float32)
    value = np.ascontiguousarray(value, dtype=np.float32)
    G = query.shape[2] // key.shape[2]
    nc = _get_nc()
    in_maps = []
    for c in range(N_CORES):
        in_maps.append({
            "query": np.ascontiguousarray(query[:, :, c * G:(c + 1) * G, :]),
            "key": np.ascontiguousarray(key[:, :, c, :]),
            "value": np.ascontiguousarray(value[:, :, c, :]),
        })
    res = run_bass_kernel_spmd(nc, in_maps, list(range(N_CORES)))
    out = np.empty_like(query)
    for c in range(N_CORES):
        out[:, :, c * G:(c + 1) * G, :] = res.results[c]["out"]
    return out



# revision 44
# speedup vs baseline: 1.0010x; 1.0010x over previous
"""v3: paired exp (one 1792-col ACT inst per q-block over a psum slot pair),
f16 PE transposes (Pool pre-casts f32->f16), DVE computes 2/16 k-tiles of exp
via 1-pass int16 Schraudolph, O lags S by 2 pairs (PT triple-buffered).

PSUM layout (f32 cols): slots 0..2 at 0/1024/2048 (S logits, rotation),
opsum bufs at 3072+{0,512} (129 cols each: 128 d + ones-column denominator).
Steady-state transposes write f16 tile slots at f32 cols 192..512 of the
opsum buf of parity (w+1)%2; bank-level PE-write/DVE-read exclusion is
enforced by gates: O(p) waits batch-(p+1) copies, window-w transposes wait
norm(w-3) and batch-(w-2) copies.
"""
import os
import numpy as np
import concourse.bass as bass
from concourse import mybir
from contextlib import ExitStack

F32 = mybir.dt.float32
F16 = mybir.dt.float16
I16 = mybir.dt.int16
I32 = mybir.dt.int32
EXP = mybir.ActivationFunctionType.Exp
SCALE = float(1.0 / np.sqrt(128.0))
LN2 = float(np.log(2.0))
A16 = (1 << 10) / LN2 * SCALE
B16 = 15.0 * (1 << 10) - 61.0
A32 = (1 << 23) / LN2 * SCALE
B32 = 127.0 * (1 << 23) - 500000.0

N_CORES = 8
N_WARM = 40
LABELS = {}


def _lab(inst, label):
    try:
        LABELS[inst.ins.name] = label
    except Exception:
        pass
    return inst


def build_attention_nc(SEQ=2048, B=2, G=4):
    NO_DUMMY = bool(int(os.environ.get("BIS_NO_DUMMY", "0")))
    NO_SPLIT = bool(int(os.environ.get("BIS_NO_SPLIT", "0")))
    NO_EXPD = bool(int(os.environ.get("BIS_NO_EXPD", "0")))
    EXPD_DUMMY = bool(int(os.environ.get("BIS_EXPD_DUMMY", "0")))
    EXPD_N = int(os.environ.get("BIS_EXPD_N", "128"))
    EXPD_NOGATE = bool(int(os.environ.get("BIS_EXPD_NOGATE", "0")))
    def expd_on(p):
        return (not NO_EXPD) and p < EXPD_N
    ACT_W = 1024 if (NO_EXPD or EXPD_DUMMY) else 768
    D = 128
    T = SEQ // 128            # 16 k/q tiles per head
    H = B * G                 # 8 (b, g) heads per core
    NPH = T                   # pairs (q-blocks) per head
    NPAIR = H * NPH           # 128
    NW = NPAIR + 2            # windows (O lags 2)
    assert T == 16 and H == 8 and B == 2

    nc = bass.Bass()
    q_ext = nc.declare_dram_parameter("query", [SEQ, B, G, D], F32, isOutput=False)
    k_ext = nc.declare_dram_parameter("key", [SEQ, B, D], F32, isOutput=False)
    v_ext = nc.declare_dram_parameter("value", [SEQ, B, D], F32, isOutput=False)
    o_ext = nc.declare_dram_parameter("out", [SEQ, B, G, D], F32, isOutput=True)

    # loads in first-use order: K(b0), Q(h0..h3), K(b1), Q(h4..h7)
    loads = [("K", 0, None)] + [("Q", 0, g) for g in range(G)]
    loads += [("K", 1, None)] + [("Q", 1, g) for g in range(G)]
    NL = len(loads)           # 10

    def q_load_index(h):
        b, g = divmod(h, G)
        return b * (G + 1) + 1 + g

    # steady-state transpose windows per load (prologue covers l0 all + l1 t0)
    due_w = {}
    per_w = {}

    trs_in_w = {w: [] for w in range(NW)}
    for i, t in enumerate(range(1, T)):       # l1 (Q h0) tiles 1..15
        trs_in_w[i // 2].append((1, t))
    for ld, ws in due_w.items():
        n = per_w.get(ld, 2)
        t = 0
        for w in ws:
            for _ in range(n):
                if t < T:
                    trs_in_w[w].append((ld, t))
                    t += 1
        assert t >= T, (ld, t)
    for w, trs in trs_in_w.items():
        assert len(trs) <= 5

    # cast (Pool f32->f16) events: (load, t0, t1, sem_load threshold)
    # (load, t0, t1, chunk): chunk 0/1 for split loads 0,1; else 0
    if NO_SPLIT:
        cast_events = [(ld, 0, 16, 0) for ld in range(NL)]
    else:
        cast_events = [(0, 0, 8, 0), (1, 0, 1, 0), (0, 8, 16, 1),
                       (1, 1, 8, 0), (1, 8, 16, 1)]
        cast_events += [(ld, 0, 16, 0) for ld in range(2, NL)]

    def cast_val_for(ld, t):
        for i, (l, t0, t1, _c) in enumerate(cast_events):
            if l == ld and t0 <= t < t1:
                return i + 1
        raise AssertionError((ld, t))

    # ---------------- schedule walk (mirrors emission exactly) ----------
    pe = 0
    pe_after_S = {}
    pe_after_O = {}
    pe_after_tr = {}
    load_last_pe = {}

    def note_tr(ld, t):
        nonlocal pe
        pe += 1
        pe_after_tr[(ld, t)] = pe
        load_last_pe[ld] = max(load_last_pe.get(ld, 0), pe)

    # prologue: l0 t0-7 -> bank6 s0-7; l1 t0 -> bank7 s0; S(0);
    # l0 t8-15 -> bank7 s1-7 + bank6 s0; S(1); then windows w>=0 trs/O.
    for t in range(8):
        note_tr(0, t)
    note_tr(1, 0)
    pe += 8
    pe_after_S[0] = pe
    for t in range(8, 16):
        note_tr(0, t)
    pe += 8
    pe_after_S[1] = pe
    for (ld, t) in trs_in_w.get(0, []):
        note_tr(ld, t)
    for w in range(NW):
        if w < NPAIR and w >= 1:
            pe += 8
            pe_after_S[2 * w] = pe
        if w >= 1:
            for (ld, t) in trs_in_w.get(w, []):
                note_tr(ld, t)
        if w >= 2:
            pe += 16
            pe_after_O[w - 2] = pe
        if w < NPAIR and w >= 1:
            pe += 8
            pe_after_S[2 * w + 1] = pe

    # DVE stream walk
    dve = 0
    copy_done = {}
    copy_batch_done = {}
    dve_exp_done = {}
    tsa_done = {}
    tsb_done = {}
    recips_done = {}
    mults_done = {}
    dve_ops = []

    def batch_runs(w):
        trs = trs_in_w.get(w, [])
        runs = []
        for (ld, t) in trs:
            if runs and runs[-1][0] == ld and runs[-1][2] == t:
                runs[-1] = [ld, runs[-1][1], t + 1]
            else:
                runs.append([ld, t, t + 1])
        return runs

    def note_copy_batch(w):
        nonlocal dve
        trs = trs_in_w.get(w, [])
        if not trs:
            copy_batch_done[w] = 0
            return
        runs = batch_runs(w)
        dve += len(runs)
        for (ld, t) in trs:
            copy_done[(ld, t)] = dve
        copy_batch_done[w] = dve
        dve_ops.append(("copies", w, runs))

    for kind, keys in [("b6a", [(0, t) for t in range(4)]),
                       ("l1t0", [(1, 0)]),
                       ("b6b", [(0, t) for t in range(4, 8)]),
                       ("b7a", [(0, t) for t in range(8, 15)]),
                       ("l0t15", [(0, 15)])]:
        dve += 1
        for kk in keys:
            copy_done[kk] = dve
        dve_ops.append(("pcopy", kind))

    for w in range(NW):
        if w < NPAIR and expd_on(w):
            dve += 1
            tsa_done[w] = dve
            dve_ops.append(("expts", w))
        note_copy_batch(w)
        if w >= 2:
            p = w - 2
            dve += 1
            recips_done[p] = dve
            dve_ops.append(("recip", p))
            dve += 1
            mults_done[p] = dve
            dve_ops.append(("mult", p))
        if w < NPAIR and expd_on(w):
            dve += 1
            tsb_done[w] = dve
            dve_exp_done[w] = dve
            dve_ops.append(("expcp", w))

    head_ready = {}

    # PT half mapping: halves ordered by ascending psum slot address
    pt_half = {}
    for p in range(NPAIR):
        if p % 3 == 1:     # slots (2,0) -> ascending (0,2): half0 = group 2p+1
            pt_half[2 * p] = 1
            pt_half[2 * p + 1] = 0
        else:
            pt_half[2 * p] = 0
            pt_half[2 * p + 1] = 1

    # ---------------- tensors ----------------
    ident = nc.alloc_sbuf_tensor("ident", [128, 128], F16)
    warm = nc.alloc_sbuf_tensor("warm", [128, 128], F16)
    bias0 = nc.alloc_sbuf_tensor("bias0", [128, 1], F32)
    scr = nc.alloc_sbuf_tensor("scr", [128, 1], F32)
    scr32 = nc.alloc_sbuf_tensor("scr32", [128, 1024], I32)
    scrPT = nc.alloc_sbuf_tensor("scrPT", [128, 512], F16)
    qnat = [nc.alloc_sbuf_tensor(f"qnat{i}", [128, T * 128], F32) for i in range(3)]
    qnat16 = [nc.alloc_sbuf_tensor(f"qnat16_{i}", [128, T * 128], F16)
              for i in range(3)]
    KT = [nc.alloc_sbuf_tensor(f"KT{b}", [128, T * 128], F16) for b in range(B)]
    QT = [nc.alloc_sbuf_tensor(f"QT{h}", [128, T * 128], F16) for h in range(H)]
    VT = [nc.alloc_sbuf_tensor(f"VT{b}", [128, T * 132], F16) for b in range(B)]
    PT = [nc.alloc_sbuf_tensor(f"PT{s}", [128, 2048], F16) for s in range(3)]
    rsb = [nc.alloc_sbuf_tensor(f"rsb{s}", [128, 1], F32) for s in range(2)]
    OS = [nc.alloc_sbuf_tensor(f"OS{s}", [128, T * 128], F32) for s in range(2)]
    psum = nc.alloc_psum_tensor("psum", [128, 4096], F32)

    pv = psum[:, :].rearrange("p (s c) -> p s c", c=1024)
    PTI = [PT[s][:, :].bitcast(I16) for s in range(3)]
    p16 = psum[:].bitcast(F16)           # [128, 8192]

    def spsum_mm(slot, ki):
        return psum[:, slot * 1024 + ki * 128: slot * 1024 + (ki + 1) * 128]

    def opsum(buf):
        off = 3072 + buf * 512
        return psum[:, off:off + 129]

    def exp_slots(p, c0, c1):
        r = p % 3
        if r == 0:
            return pv[:, 0:2, c0:c1]
        if r == 1:
            return pv[:, 0::2, c0:c1]
        return pv[:, 1:3, c0:c1]

    def tr_parity(w):
        return (w + 1) % 2

    def tr_psum_w(w, k):
        base = (6 + tr_parity(w)) * 1024 + 384 + k * 128
        return p16[:, base:base + 128]

    PRO_SLOT = {}       # (ld,t) -> f16 col base for prologue trs
    for t in range(8):
        PRO_SLOT[(0, t)] = 6 * 1024 + t * 128
    PRO_SLOT[(1, 0)] = 7 * 1024
    for t in range(8, 15):
        PRO_SLOT[(0, t)] = 7 * 1024 + (t - 7) * 128
    PRO_SLOT[(0, 15)] = 6 * 1024

    with ExitStack() as ctx:
        sem_pe = ctx.enter_context(nc.semaphore("sem_pe"))
        sem_act = ctx.enter_context(nc.semaphore("sem_act"))
        sem_dve = ctx.enter_context(nc.semaphore("sem_dve"))
        sem_cast = ctx.enter_context(nc.semaphore("sem_cast"))
        sem_pool = ctx.enter_context(nc.semaphore("sem_pool"))
        sem_load = {}
        for i in range(NL):
            nch = 1 if NO_SPLIT else (2 if i < 2 else 1)
            for c in range(nch):
                sem_load[(i, c)] = ctx.enter_context(
                    nc.semaphore(f"sem_load{i}_{c}"))
        sem_out = [ctx.enter_context(nc.semaphore(f"sem_out{h}"))
                   for h in range(H)]
        sem_str = {i: ctx.enter_context(nc.semaphore(f"sem_str{i}"))
                   for i in range(2, NL)}
        sem_v = [ctx.enter_context(nc.semaphore(f"sem_v{b}")) for b in range(B)]
        block = ctx.enter_context(nc.Block())

        @block.sync
        def _(sync):
            def ld_src(i):
                kind, b, g = loads[i]
                return k_ext[:, b, :] if kind == "K" else q_ext[:, b, g, :]

            def emit_load(i, t0, t1, chunk):
                src = ld_src(i).rearrange("(t p) d -> p t d", p=128)
                dst = qnat[i % 3][:].rearrange("p (t d) -> p t d", d=128)
                nc.sync.dma_start(
                    out=dst[:, t0:t1, :], in_=src[:, t0:t1, :],
                ).then_inc(sem_load[(i, chunk)], 16)

            def emit_xbar(i):
                kind, b, g = loads[i]
                tt = KT[b] if kind == "K" else QT[b * G + g]
                nc.sync.wait_ge(sem_cast, cast_val_for(i, 0))
                dst = tt[:].rearrange("p (t d) -> p t d", d=128)
                _lab(nc.sync.dma_start_transpose(
                    dst[:, 0:16, :],
                    qnat16[i % 3][:, 0:T * 128],
                ), f"X(l{i})").then_inc(sem_str[i], 16)

            emit_load(0, 0, 8, 0)
            emit_load(1, 0, 8, 0)
            emit_load(0, 8, 16, 1)
            emit_load(1, 8, 16, 1)
            for i in range(2, NL):
                if i == 3 or i == 4:
                    nc.sync.wait_ge(sem_pe, load_last_pe[i - 3])
                elif i >= 5:
                    nc.sync.wait_ge(sem_str[i - 3], 16)
                emit_load(i, 0, 16, 0)
                if i >= 4:
                    emit_xbar(i - 2)
            emit_xbar(8)
            emit_xbar(9)
            for h in range(H):
                nc.sync.wait_ge(sem_out[h], 64 if h == H - 1 else 32)

        @block.gpsimd
        def _(gp):
            nc.gpsimd.memset(warm[:], 0.0).then_inc(sem_pool)
            nc.gpsimd.memset(ident[:], 0.0).then_inc(sem_pool)
            nc.gpsimd.wait_ge(sem_pool, 2)
            nc.gpsimd.affine_select(
                out=ident[:], in_=ident[:],
                compare_op=mybir.AluOpType.not_equal, fill=1.0,
                base=0, pattern=[[-1, 128]], channel_multiplier=1,
            ).then_inc(sem_pool)
            nc.gpsimd.memset(bias0[:], 0.0).then_inc(sem_pool)
            for b in range(B):
                vt3 = VT[b][:].rearrange("p (t c) -> p t c", c=132)
                nc.gpsimd.memset(vt3[:, :, 128:129], 1.0).then_inc(sem_pool)
                nc.gpsimd.dma_start(
                    out=vt3[:, :, 0:128],
                    in_=v_ext[:, b, :].rearrange("(t p) d -> p t d", p=128),
                ).then_inc(sem_v[b], 16)

            def cast(idx):
                ld, t0, t1, chunk = cast_events[idx]
                nc.gpsimd.wait_ge(sem_load[(ld, chunk)], 16)
                nc.gpsimd.tensor_copy(
                    qnat16[ld % 3][:, t0 * 128:t1 * 128],
                    qnat[ld % 3][:, t0 * 128:t1 * 128],
                ).then_inc(sem_cast)

            def out_store(h, half):
                b, g = divmod(h, G)
                oh = o_ext[:, b, g, :].rearrange("(t p) d -> p t d", p=128)
                osh = OS[h % 2][:].rearrange("p (t d) -> p t d", d=128)
                if h == H - 1:
                    for q in (0, 1) if half == 0 else (2, 3):
                        p_end = h * NPH + 4 * q + 3
                        nc.gpsimd.wait_ge(sem_dve, mults_done[p_end])
                        nc.gpsimd.dma_start(
                            out=oh[:, 4 * q:4 * q + 4, :],
                            in_=osh[:, 4 * q:4 * q + 4, :],
                        ).then_inc(sem_out[h], 16)
                    return
                hf = T // 2
                p_end = h * NPH + (half + 1) * hf - 1
                nc.gpsimd.wait_ge(sem_dve, mults_done[p_end])
                nc.gpsimd.dma_start(
                    out=oh[:, half * hf:(half + 1) * hf, :],
                    in_=osh[:, half * hf:(half + 1) * hf, :],
                ).then_inc(sem_out[h], 16)

            # c0..c4: l0/l1 chunks; c5..c12: loads 2..9
            if NO_SPLIT:
                order = ["c0", "c1", "c2", "c3", "c4", "o0a", "c5",
                         "o0b", "o1a", "c6", "o1b", "o2a", "c7", "o2b",
                         "o3a", "c8", "o3b", "c9", "o4a", "o4b",
                         "o5a", "o5b", "o6a", "o6b", "o7a", "o7b"]
            else:
                order = ["c0", "c1", "c2", "c3", "c4", "c5", "c6",
                         "c7", "o0a", "c8", "o0b", "o1a", "c9", "o1b",
                         "o2a", "c10", "o2b", "o3a", "c11", "o3b",
                         "c12", "o4a", "o4b", "o5a", "o5b",
                         "o6a", "o6b", "o7a", "o7b"]
            for op in order:
                if op[0] == "c":
                    cast(int(op[1:]))
                else:
                    out_store(int(op[1]), 0 if op[2] == "a" else 1)

        @block.tensor
        def _(te):
            done_str = set()

            def str_wait(i):
                if i not in done_str:
                    done_str.add(i)
                    nc.tensor.wait_ge(sem_str[i], 16)

            nc.tensor.wait_ge(sem_pool, 1)
            for i in range(N_WARM):
                _lab(nc.tensor.matmul(
                    psum[:, 2048:2176], warm[:], warm[:],
                    start=True, stop=True, skip_group_check=True,
                ), f"warm{i}")
            nc.tensor.wait_ge(sem_pool, 3)
            seen_cast = set()
            last_dve_wait = [0]

            def twait(val):
                if val > last_dve_wait[0]:
                    last_dve_wait[0] = val
                    nc.tensor.wait_ge(sem_dve, val)

            def emit_tr(ld, t, dst):
                cv = cast_val_for(ld, t)
                if cv not in seen_cast:
                    seen_cast.add(cv)
                    nc.tensor.wait_ge(sem_cast, cv)
                _lab(nc.tensor.transpose(
                    dst, qnat16[ld % 3][:, t * 128:(t + 1) * 128], ident[:],
                ), f"tr(l{ld},t{t})").then_inc(sem_pe)

            def emit_S(g):
                p = g >> 1
                h = p // NPH
                slot = g % 3
                kp = g & 1
                b = h // G
                qc = p % NPH
                if g == 2 * h * NPH and h >= 1:
                    if h >= G:
                        str_wait(5)
                    str_wait(q_load_index(h))
                if h == 0 and qc > 0 and kp == 0:
                    twait(copy_done[(1, qc)])
                if g == 0:
                    twait(copy_done[(1, 0)])
                if g == 1:
                    twait(copy_done[(0, 14)])
                for ki in range(8):
                    kt = kp * 8 + ki
                    inst = nc.tensor.matmul(
                        spsum_mm(slot, ki),
                        KT[b][:, kt * 128:(kt + 1) * 128],
                        QT[h][:, qc * 128:(qc + 1) * 128],
                        start=True, stop=True, skip_group_check=True,
                    )
                    if ki == 0 and g >= 3:
                        inst._wait_ge(sem_act, g - 2)
                    if g == 0 and ki == 4:
                        inst._wait_ge(sem_dve, copy_done[(0, 4)])
                    if g == 1 and ki == 7:
                        inst._wait_ge(sem_dve, copy_done[(0, 15)])
                    _lab(inst, f"S(g{g},ki{ki})")
                    inst.then_inc(sem_pe)

            def emit_O(p):
                h = p // NPH
                b = h // G
                buf = p % 2
                if p == 0 or p == G * NPH:
                    nc.tensor.wait_ge(sem_v[b], 16)
                    nc.tensor.wait_ge(sem_pool, 5 + b)
                w_gate = 0
                if p >= 2:
                    w_gate = mults_done[p - 2]        # opsum buf reuse
                w_gate = max(w_gate, copy_batch_done.get(p + 1, 0))  # bank P10
                if expd_on(p):
                    w_gate = max(w_gate, tsb_done[p])
                if expd_on(p + 1):
                    w_gate = max(w_gate, tsa_done[p + 1])
                if w_gate:
                    twait(w_gate)
                vt3 = VT[b][:].rearrange("p (t c) -> p t c", c=132)
                kts = [0, 1, 2, 3, 4] + list(range(8, 16)) + [5, 6, 7]
                for i, kt in enumerate(kts):
                    g = 2 * p + (kt // 8)
                    half = pt_half[g]
                    ki = kt % 8
                    inst = nc.tensor.matmul(
                        opsum(buf),
                        PT[p % 3][:, half * 1024 + ki * 128:
                                  half * 1024 + (ki + 1) * 128],
                        vt3[:, kt, 0:129],
                        start=(i == 0), stop=(i == len(kts) - 1),
                        skip_group_check=True,
                    )
                    if i == 0:
                        inst._wait_ge(sem_act, 2 * p + 2)
                    _lab(inst, f"O(p{p},kt{kt})")
                    inst.then_inc(sem_pe)

            # prologue
            for t in range(8):
                emit_tr(0, t, p16[:, PRO_SLOT[(0, t)]:PRO_SLOT[(0, t)] + 128])
            emit_tr(1, 0, p16[:, PRO_SLOT[(1, 0)]:PRO_SLOT[(1, 0)] + 128])
            emit_S(0)
            for t in range(8, 16):
                if t == 15:
                    twait(copy_done[(0, 0)])   # bank6 s0 reuse
                emit_tr(0, t, p16[:, PRO_SLOT[(0, t)]:PRO_SLOT[(0, t)] + 128])
            emit_S(1)
            for k, (ld, t) in enumerate(trs_in_w.get(0, [])):
                emit_tr(ld, t, tr_psum_w(0, k))

            for w in range(NW):
                if w < NPAIR and w >= 1:
                    emit_S(2 * w)
                trs = trs_in_w.get(w, []) if w >= 1 else []
                if trs:
                    twait(copy_done[(0, 15)])          # all prologue copies
                    if w >= 3:
                        twait(mults_done[w - 3])       # norm done (same bank)
                    if w >= 2:
                        twait(copy_batch_done.get(w - 2, 0))  # slot reuse
                    for k, (ld, t) in enumerate(trs):
                        emit_tr(ld, t, tr_psum_w(w, k))
                if w >= 2:
                    emit_O(w - 2)
                if w < NPAIR and w >= 1:
                    emit_S(2 * w + 1)

        @block.scalar
        def _(sc):
            nc.scalar.wait_ge(sem_pool, 4)
            if not NO_DUMMY:
                nc.scalar.activation(                  # preload Exp table
                    out=scr[:, 0:1], in_=bias0[:, 0:1],
                    func=EXP, bias=bias0[:, 0:1], scale=1.0,
                )
            for p in range(NPAIR):
                ov = PT[p % 3][:, :].rearrange("p (s c) -> p s c", c=1024)
                for pos in range(2):
                    g = 2 * p + pos
                    slot = g % 3
                    half = pt_half[g]
                    aw = 512 if (expd_on(p) and pos == 0) else 1024
                    _lab(nc.scalar.activation(
                        out=ov[:, half:half + 1, 0:aw],
                        in_=pv[:, slot:slot + 1, 0:aw],
                        func=EXP, bias=bias0[:, 0:1], scale=SCALE,
                    )._wait_ge(sem_pe, pe_after_S[g]),
                        f"exp(g{g})").then_inc(sem_act)

        @block.vector
        def _(ve):
            def emit_pcopy(kind):
                if kind == "b6a":
                    nc.vector.wait_ge(sem_pe, pe_after_tr[(0, 3)])
                    nc.vector.tensor_copy(
                        KT[0][:, 0:512],
                        p16[:, 6 * 1024:6 * 1024 + 512]).then_inc(sem_dve)
                elif kind == "b6b":
                    nc.vector.wait_ge(sem_pe, pe_after_tr[(0, 7)])
                    nc.vector.tensor_copy(
                        KT[0][:, 512:1024],
                        p16[:, 6 * 1024 + 512:6 * 1024 + 1024]).then_inc(sem_dve)
                elif kind == "l1t0":
                    nc.vector.wait_ge(sem_pe, pe_after_tr[(1, 0)])
                    nc.vector.tensor_copy(
                        QT[0][:, 0:128],
                        p16[:, 7 * 1024:7 * 1024 + 128]).then_inc(sem_dve)
                elif kind == "b7a":
                    nc.vector.wait_ge(sem_pe, pe_after_tr[(0, 14)])
                    nc.vector.tensor_copy(
                        KT[0][:, 1024:1920],
                        p16[:, 7 * 1024 + 128:8 * 1024]).then_inc(sem_dve)
                else:  # l0t15
                    nc.vector.wait_ge(sem_pe, pe_after_tr[(0, 15)])
                    nc.vector.tensor_copy(
                        KT[0][:, 1920:2048],
                        p16[:, 6 * 1024:6 * 1024 + 128]).then_inc(sem_dve)

            def emit_copies(w, runs):
                trs = trs_in_w[w]
                nc.vector.wait_ge(sem_pe, pe_after_tr[tuple(trs[-1])])
                k = 0
                base = (6 + tr_parity(w)) * 1024 + 384
                for (ld, t0, t1) in runs:
                    n = t1 - t0
                    src = p16[:, base + k * 128:base + (k + n) * 128]
                    kind, b, g = loads[ld]
                    tt = KT[b] if kind == "K" else QT[b * G + g]
                    _lab(nc.vector.tensor_copy(
                        tt[:, t0 * 128:t1 * 128], src), f"cp(w{w},l{ld},t{t0}-{t1})").then_inc(sem_dve)
                    k += n

            def emit_expts(p):
                g = 2 * p
                slot = g % 3
                nc.vector.wait_ge(sem_pe, pe_after_S[g])
                off = (p % 2) * 512
                _lab(nc.vector.tensor_scalar(
                    scr32[:, off:off + 512], pv[:, slot, 512:1024],
                    A32, B32, op0=mybir.AluOpType.mult,
                    op1=mybir.AluOpType.add,
                ), f"expDts({p})").then_inc(sem_dve)

            def emit_expcp(p):
                half = pt_half[2 * p]
                off = (p % 2) * 512
                nc.vector.wait_ge(sem_dve, tsa_done[p])   # scr32 RAW drain
                _lab(nc.vector.tensor_copy(
                    PT[p % 3][:, half * 1024 + 512:half * 1024 + 1024],
                    scr32[:, off:off + 512].bitcast(F32),
                ), f"expDcp({p})").then_inc(sem_dve)

            def emit_recip(p):
                buf = p % 2
                nc.vector.wait_ge(sem_pe, pe_after_O[p])
                if p >= 2:
                    nc.vector.wait_ge(sem_dve, mults_done[p - 2])
                _lab(nc.vector.reciprocal(
                    rsb[buf][:, 0:1], opsum(buf)[:, 128:129]), f"recip({p})").then_inc(sem_dve)

            def emit_mult(p):
                h = p // NPH
                qc = p % NPH
                buf = p % 2
                nc.vector.wait_ge(sem_dve, recips_done[p])
                if qc == 0 and h >= 2:
                    nc.vector.wait_ge(sem_out[h - 2], 32)
                _lab(nc.vector.tensor_scalar(
                    OS[h % 2][:, qc * 128:(qc + 1) * 128],
                    opsum(buf)[:, 0:128],
                    rsb[buf][:, 0:1],
                    None,
                    op0=mybir.AluOpType.mult,
                ), f"mult({p})").then_inc(sem_dve)

            n_pad = int(os.environ.get("BIS_DVE_PAD", "0"))
            pad_kind = os.environ.get("BIS_PAD_KIND", "tiny")
            for i in range(n_pad):
                if pad_kind == "tiny":
                    nc.vector.tensor_copy(scr[:, 0:1], bias0[:, 0:1])
                elif pad_kind == "ts_psum":
                    nc.vector.tensor_scalar(
                        scr32[:, :].rearrange("p (s c) -> p s c", c=256)[:, 0:2, :],
                        pv[:, 0::2, 768:1024],
                        A32, B32, op0=mybir.AluOpType.mult,
                        op1=mybir.AluOpType.add)
                elif pad_kind == "ts_sbuf":
                    nc.vector.tensor_scalar(
                        scr32[:, 0:256], scrPT[:, 0:256].bitcast(F32).to_broadcast([128, 256]) if False else scr32[:, 256:512].bitcast(F32),
                        A32, B32, op0=mybir.AluOpType.mult,
                        op1=mybir.AluOpType.add)
                elif pad_kind == "cp_big":
                    nc.vector.tensor_copy(
                        scrPT[:, :].rearrange("p (s c) -> p s c", c=256)[:, 0:2, :],
                        scr32[:].bitcast(F32).rearrange(
                            "p (s c) -> p s c", c=256)[:, 0:2, :])
            for op in dve_ops:
                if op[0] == "pcopy":
                    emit_pcopy(op[1])
                elif op[0] == "copies":
                    emit_copies(op[1], op[2])
                elif op[0] == "expts":
                    emit_expts(op[1])
                elif op[0] == "expcp":
                    emit_expcp(op[1])
                elif op[0] == "recip":
                    emit_recip(op[1])
                else:
                    emit_mult(op[1])

    return nc


_NC = None


def _get_nc():
    global _NC
    if _NC is None:
        _NC = build_attention_nc(2048, 2, 4)
    return _NC


def kernel(query, key, value):
    from concourse.bass_utils import run_bass_kernel_spmd

    query = np.ascontiguousarray(query, dtype=np.float32)
    key = np.ascontiguousarray(key, dtype=np.float32)
    value = np.ascontiguousarray(value, dtype=np.float32)
    G = query.shape[2] // key.shape[2]
    nc = _get_nc()
    in_maps = []
    for c in range(N_CORES):
        in_maps.append({
            "query": np.ascontiguousarray(query[:, :, c * G:(c + 1) * G, :]),
            "key": np.ascontiguousarray(key[:, :, c, :]),
            "value": np.ascontiguousarray(value[:, :, c, :]),
        })
    res = run_bass_kernel_spmd(nc, in_maps, list(range(N_CORES)))
    out = np.empty_like(query)
    for c in range(N_CORES):
        out[:, :, c * G:(c + 1) * G, :] = res.results[c]["out"]
    return out

